# revision 17
# baseline (speedup 1.0000x reference)
"""Trainium2 Bass kernel for nn_Evolution (gated recurrence, T=64, N=2048, DR2=512).

Math per timestep t:
    mm_t  = concat([h_{t-1}, static_t], -1) @ w1          # [N, 512]
    h_t   = sigmoid(mm_t * thr_t + h_{t-1} * (1 - thr_t)) * exp(-1/2)

Device strategy (per core, N sharded 8 ways -> 256 rows/core), layout
[feature (partitions), row (free)]:
  - Host pre-scales static_t by thr_t (row-wise) and transposes, so the PSUM
    accumulation  z_t = thr*mm + (1-thr)*h  is built ENTIRELY on the
    TensorEngine:
        z_t = (thr_t*static_t) @ w1b            (host-prescaled input)
            + u_t @ (w1a - I)                    u_t = DECAY*thr_t . s_{t-1}
            + (DECAY*I) @ s_{t-1}                identity-matmul blend
    (the -I term of the blend is folded into the w1a weights on the host);
    s_t = sigmoid(z_t) (ACT reads PSUM directly) and h_t = DECAY*s_t.
  - Matmuls run as float32r (1 cyc/row on TRN2 vs 4 for fp32).
  - Static matmuls are batched across step PAIRS (moving free dim 512); PSUM
    tiles are [128, j, t_pair, 256] two-bank tiles so each matmul stays
    within one bank.
  - DVE computes u (scalar_tensor_tensor), h (tensor_scalar 2x mode) and diff.
  - DRAM staging is partition-major so every DMA moves 2-8KB contiguous
    runs per partition.
"""
import os
import sys

sys.path.insert(0, "/opt/trn_rl_repo")
import numpy as np

import concourse.bacc as bacc
import concourse.bass as bass
import concourse.mybir as mybir
from concourse.tile import TileContext
from concourse.bass_utils import run_bass_kernel_spmd

T, N, DR2 = 64, 2048, 512
CORES = 8
RPC = N // CORES  # 256 rows per core
PAIRS = T // 2
DECAY = float(np.exp(-0.5))
F32 = mybir.dt.float32
F32R = mybir.dt.float32r if os.environ.get("KERNEL_MM_DT", "f32r") == "f32r" else mybir.dt.float32
PAIR_AHEAD = 3  # static-pair DMA prefetch depth

_prog = None  # cached Bass program


def build_program():
    nc = bacc.Bacc()

    # partition-major staging layouts (host pre-arranged)
    staticPM = nc.dram_tensor("staticPM", [PAIRS, 128, 4 * 2 * RPC], F32R, kind="ExternalInput")
    thrbc = nc.dram_tensor("thrbc", [T, 128, RPC], F32, kind="ExternalInput")
    u0T = nc.dram_tensor("u0T", [DR2, RPC], F32R, kind="ExternalInput")
    h0T = nc.dram_tensor("h0T", [DR2, RPC], F32R, kind="ExternalInput")
    w1a = nc.dram_tensor("w1a", [128, 4 * 4 * 128], F32R, kind="ExternalInput")
    w1b = nc.dram_tensor("w1b", [128, 4 * 4 * 128], F32R, kind="ExternalInput")
    idents = nc.dram_tensor("idents", [128, 3, 128], F32R, kind="ExternalInput")

    hdPM = nc.dram_tensor("hdPM", [T - 1, 128, 2 * 4 * RPC], F32, kind="ExternalOutput")

    with TileContext(nc) as tc:
        with (
            tc.tile_pool(name="wp", bufs=1) as wp,
            tc.tile_pool(name="stp", bufs=PAIR_AHEAD + 1) as stp,
            tc.tile_pool(name="thp", bufs=4) as thp,
            tc.tile_pool(name="sp", bufs=3) as sp,
            tc.tile_pool(name="up", bufs=2) as up,
            tc.tile_pool(name="hp", bufs=3) as hp,
            tc.tile_pool(name="dp", bufs=2) as dp,
            tc.tile_pool(name="pzp", bufs=2, space="PSUM") as pzp,
        ):
            # ---- persistent weights / constants ----
            # w1b first: the very first static matmuls need only w1b + st0
            w1b_t = wp.tile([128, 4, 4, 128], F32R)
            nc.sync.dma_start(
                out=w1b_t[:], in_=w1b.rearrange("p (m k j) -> p m k j", m=4, k=4)
            )


            pz = {}  # pair -> [pz2_g0, pz2_g1], each [128, j, t, 256] (2 banks)
            st_tiles = {}
            thr_tiles = {}

            def dma_static(p):
                st = stp.tile([128, 4, 2, RPC], F32R, name=f"st{p}", tag="static")
                nc.sync.dma_start(
                    out=st[:],
                    in_=staticPM[p].rearrange("p (k t n) -> p k t n", k=4, t=2),
                )
                st_tiles[p] = st

            def dma_thr(t):
                thrB = thp.tile([128, 1, 1, RPC], F32, name=f"thrB{t}", tag="thr")
                nc.scalar.dma_start(out=thrB[:, 0, 0, :], in_=thrbc[t])
                thr_tiles[t] = thrB

            def mm_static_pair(p):
                """16 N=512 w1b matmuls covering steps (2p, 2p+1)."""
                st = st_tiles.pop(p)
                pzt = pzp.tile(
                    [128, 2, 2, 2, RPC], F32, name=f"pz{p}", tag="pz"
                )
                pz[p] = pzt
                for g in range(2):
                    for j in range(2):
                        m = 2 * g + j
                        for k in range(4):
                            nc.tensor.matmul(
                                pzt[:, g, j, :, :],
                                w1b_t[:, m, k, :],
                                st[:, k, :, :],
                                start=(k == 0),
                                stop=False,
                            )

            def u_slice(u_t, k):
                return u_t[:, k // 2, k % 2, :]

            def emit_gate(t, s_t, u_t):
                """Identity blend + dynamic matmuls for step t into pz[t//2]."""
                pzt = pz[t // 2]
                ti = t % 2
                id_s = id_t[:, 2, :] if t == 0 else id_t[:, 0, :]
                for g in range(2):
                    for j in range(2):
                        nc.tensor.matmul(
                            pzt[:, g, j, ti, :],
                            id_s,
                            s_t[:, g, j, :],
                            start=False,
                            stop=False,
                        )
                for g in range(2):
                    for j in range(2):
                        m = 2 * g + j
                        for k in range(4):
                            nc.tensor.matmul(
                                pzt[:, g, j, ti, :],
                                w1a_t[:, m, k, :],
                                u_slice(u_t, k),
                                start=False,
                                stop=(ti == 1 and k == 3),
                            )

            # ---- prologue (order matters: w1b+st0 gate the first matmuls) ----
            dma_static(0)

            # step-0 gate inputs
            u0_t = wp.tile([128, 2, 2, 256], F32R, name="u0t")
            nc.sync.dma_start(
                out=u0_t[:],
                in_=u0T.rearrange("(g j p) n -> p g j n", p=128, j=2),
            )
            h0_t = wp.tile([128, 2, 2, 256], F32R, name="h0t")
            nc.sync.dma_start(
                out=h0_t[:],
                in_=h0T.rearrange("(g j p) n -> p g j n", p=128, j=2),
            )
            id_t = wp.tile([128, 3, 128], F32R)
            nc.sync.dma_start(out=id_t[:], in_=idents[:])
            w1a_t = wp.tile([128, 4, 4, 128], F32R)
            nc.sync.dma_start(
                out=w1a_t[:], in_=w1a.rearrange("p (m k j) -> p m k j", m=4, k=4)
            )
            for p in range(1, PAIR_AHEAD):
                dma_static(p)
            for t in range(1, 3):
                dma_thr(t)
            mm_static_pair(0)
            emit_gate(0, h0_t, u0_t)

            h_prev = None
            for t in range(T):
                if t % 2 == 1:
                    p_new = (t + 1) // 2
                    if p_new + PAIR_AHEAD - 1 < PAIRS:
                        dma_static(p_new + PAIR_AHEAD - 1)
                    # next pair's static matmuls fill the sigma/u PE gap
                    if p_new < PAIRS:
                        mm_static_pair(p_new)
                if t + 3 < T:
                    dma_thr(t + 3)

                # sigma_t straight out of PSUM (strided read across 4 banks)
                s_t = sp.tile([128, 2, 2, 256], F32R, name=f"s{t}", tag="s")
                nc.scalar.activation(
                    s_t[:].rearrange("p g j n -> p (g j) n"),
                    pz[t // 2][:, :, :, t % 2, :].rearrange("p g j n -> p (g j) n"),
                    mybir.ActivationFunctionType.Sigmoid,
                )

                # next step's gate: u_{t+1} = (s_t * DECAY) * thr_{t+1}
                if t + 1 < T:
                    thrB = thr_tiles.pop(t + 1)
                    u_t = up.tile([128, 2, 2, 256], F32R, name=f"u{t}", tag="u")
                    nc.vector.scalar_tensor_tensor(
                        out=u_t[:].rearrange("p g j n -> p (g j) n"),
                        in0=s_t[:].rearrange("p g j n -> p (g j) n"),
                        scalar=DECAY,
                        in1=thrB[:, 0, :, :].to_broadcast((128, 4, RPC)),
                        op0=mybir.AluOpType.mult,
                        op1=mybir.AluOpType.mult,
                    )
                    emit_gate(t + 1, s_t, u_t)

                # outputs: hd[:,0] = h_t = DECAY*s_t ; hd[:,1] = h_t - h_{t-1}
                hd_t = hp.tile([128, 2, 2, 2, 256], F32, name=f"hd{t}", tag="hd")
                nc.vector.tensor_scalar_mul(
                    hd_t[:, 0].rearrange("p g j n -> p (g j) n"),
                    s_t[:].rearrange("p g j n -> p (g j) n"),
                    DECAY,
                )
                if t >= 1:
                    nc.vector.scalar_tensor_tensor(
                        out=hd_t[:, 1].rearrange("p g j n -> p (g j) n"),
                        in0=s_t[:].rearrange("p g j n -> p (g j) n"),
                        scalar=DECAY,
                        in1=h_prev[:, 0].rearrange("p g j n -> p (g j) n"),
                        op0=mybir.AluOpType.mult,
                        op1=mybir.AluOpType.subtract,
                    )
                    nc.sync.dma_start(
                        out=hdPM[t - 1].rearrange(
                            "p (w g j n) -> p w g j n", w=2, g=2, j=2
                        ),
                        in_=hd_t[:],
                    )
                h_prev = hd_t

    nc.compile()
    return nc


def get_program():
    global _prog
    if _prog is None:
        _prog = build_program()
    return _prog


def kernel(all_data_static, threshold_nc, all_data_dynamic_now, w1):
    all_data_static = np.asarray(all_data_static, dtype=np.float32)
    threshold_nc = np.asarray(threshold_nc, dtype=np.float32)
    h0 = np.asarray(all_data_dynamic_now, dtype=np.float32)
    w1 = np.asarray(w1, dtype=np.float32)

    nc = get_program()

    idents = np.zeros((128, 3, 128), dtype=np.float32)
    eye = np.eye(128, dtype=np.float32)
    idents[:, 0, :] = DECAY * eye
    idents[:, 1, :] = -eye
    idents[:, 2, :] = eye
    def wpm(w):
        # [512(k-major), 512(m-major)] -> [128(p), 4(m), 4(k), 128(j)]
        return np.ascontiguousarray(
            w.reshape(4, 128, 4, 128).transpose(1, 2, 0, 3)
        ).reshape(128, 4 * 4 * 128)

    w1a = wpm(w1[:DR2] - np.eye(DR2, dtype=np.float32))
    w1b = wpm(w1[DR2:])

    in_maps = []
    for c in range(CORES):
        r0 = c * RPC
        sl = slice(r0, r0 + RPC)
        thr_c = threshold_nc[:, sl, 0]  # [T, RPC]
        thr_bc = np.ascontiguousarray(
            np.broadcast_to(thr_c[:, None, :], (T, 128, RPC))
        )
        static_c = all_data_static[:, sl, :]  # [T, RPC, DR2]
        staticT = (static_c * thr_c[:, :, None]).transpose(0, 2, 1)  # [T, DR2, RPC]
        # partition-major pairs: [PAIRS, 128(p), 4(k), 2(t), RPC]
        staticPM = np.ascontiguousarray(
            staticT.reshape(PAIRS, 2, 4, 128, RPC).transpose(0, 3, 2, 1, 4)
        ).reshape(PAIRS, 128, 4 * 2 * RPC)
        h0_c = h0[sl]  # [RPC, DR2]
        u0T = np.ascontiguousarray((thr_c[0][:, None] * h0_c).T)
        h0Tc = np.ascontiguousarray(h0_c.T)
        in_maps.append(
            {
                "staticPM": staticPM,
                "thrbc": thr_bc,
                "u0T": u0T,
                "h0T": h0Tc,
                "w1a": w1a,
                "w1b": w1b,
                "idents": idents,
            }
        )

    trace = os.environ.get("KERNEL_TRACE", "0") == "1"
    res = run_bass_kernel_spmd(
        nc,
        in_maps,
        core_ids=list(range(CORES)),
        trace=trace,
        trace_cores=[0] if trace else None,
    )
    kernel.last_exec_time_ns = res.exec_time_ns
    kernel.last_results = res

    dynamic = np.empty((T, N, DR2), dtype=np.float32)
    diff = np.empty((T - 1, N, DR2), dtype=np.float32)
    dynamic[0] = h0
    for c in range(CORES):
        sl = slice(c * RPC, (c + 1) * RPC)
        out = res.results[c]
        # [63, 128(p), 2(w), 4(c), RPC] -> w-split -> [63, RPC, 4(c), 128(p)]
        hd = out["hdPM"].reshape(T - 1, 128, 2, 4, RPC)
        dynamic[1:, sl, :] = hd[:, :, 0].transpose(0, 3, 2, 1).reshape(T - 1, RPC, DR2)
        diff[:, sl, :] = hd[:, :, 1].transpose(0, 3, 2, 1).reshape(T - 1, RPC, DR2)
    final = dynamic[-1].copy()
    return dynamic, final, diff


# revision 18
# speedup vs baseline: 1.0635x; 1.0635x over previous
"""Trainium2 Bass kernel for nn_Evolution (gated recurrence, T=64, N=2048, DR2=512).

Math per timestep t:
    mm_t  = concat([h_{t-1}, static_t], -1) @ w1          # [N, 512]
    h_t   = sigmoid(mm_t * thr_t + h_{t-1} * (1 - thr_t)) * exp(-1/2)

Device strategy (per core, N sharded 8 ways -> 256 rows/core), layout
[feature (partitions), row (free)]:
  - Host pre-scales static_t by thr_t (row-wise) and transposes, so the PSUM
    accumulation  z_t = thr*mm + (1-thr)*h  is built ENTIRELY on the
    TensorEngine:
        z_t = (thr_t*static_t) @ w1b            (host-prescaled input)
            + u_t @ (w1a - I)                    u_t = DECAY*thr_t . s_{t-1}
            + (DECAY*I) @ s_{t-1}                identity-matmul blend
    (the -I term of the blend is folded into the w1a weights on the host);
    s_t = sigmoid(z_t) (ACT reads PSUM directly) and h_t = DECAY*s_t.
  - Matmuls run as float32r (1 cyc/row on TRN2 vs 4 for fp32).
  - Static matmuls are batched across step PAIRS (moving free dim 512); PSUM
    tiles are [128, j, t_pair, 256] two-bank tiles so each matmul stays
    within one bank.
  - DVE computes u (scalar_tensor_tensor), h (tensor_scalar 2x mode) and diff.
  - DRAM staging is partition-major so every DMA moves 2-8KB contiguous
    runs per partition.
"""
import os
import sys

sys.path.insert(0, "/opt/trn_rl_repo")
import numpy as np

import concourse.bacc as bacc
import concourse.bass as bass
import concourse.mybir as mybir
from concourse.tile import TileContext
from concourse.bass_utils import run_bass_kernel_spmd

T, N, DR2 = 64, 2048, 512
CORES = 8
RPC = N // CORES  # 256 rows per core
PAIRS = T // 2
DECAY = float(np.exp(-0.5))
F32 = mybir.dt.float32
F32R = mybir.dt.float32r if os.environ.get("KERNEL_MM_DT", "f32r") == "f32r" else mybir.dt.float32
PAIR_AHEAD = 3  # static-pair DMA prefetch depth

_prog = None  # cached Bass program


def build_program():
    nc = bacc.Bacc()

    # partition-major staging layouts (host pre-arranged)
    staticPM = nc.dram_tensor("staticPM", [PAIRS, 128, 4 * 2 * RPC], F32R, kind="ExternalInput")
    thrbc = nc.dram_tensor("thrbc", [T, 128, RPC], F32, kind="ExternalInput")
    u0T = nc.dram_tensor("u0T", [DR2, RPC], F32R, kind="ExternalInput")
    h0T = nc.dram_tensor("h0T", [DR2, RPC], F32R, kind="ExternalInput")
    w1a = nc.dram_tensor("w1a", [128, 4 * 4 * 128], F32R, kind="ExternalInput")
    w1b = nc.dram_tensor("w1b", [128, 4 * 4 * 128], F32R, kind="ExternalInput")
    idents = nc.dram_tensor("idents", [128, 3, 128], F32R, kind="ExternalInput")

    hdPM = nc.dram_tensor("hdPM", [T - 1, 128, 2 * 4 * RPC], F32, kind="ExternalOutput")

    with TileContext(nc) as tc:
        with (
            tc.tile_pool(name="wp", bufs=1) as wp,
            tc.tile_pool(name="stp", bufs=PAIR_AHEAD + 1) as stp,
            tc.tile_pool(name="thp", bufs=4) as thp,
            tc.tile_pool(name="sp", bufs=3) as sp,
            tc.tile_pool(name="up", bufs=2) as up,
            tc.tile_pool(name="hp", bufs=3) as hp,
            tc.tile_pool(name="dp", bufs=2) as dp,
            tc.tile_pool(name="pzp", bufs=2, space="PSUM") as pzp,
        ):
            # ---- persistent weights / constants ----
            # w1b first: the very first static matmuls need only w1b + st0
            w1b_t = wp.tile([128, 4, 4, 128], F32R)
            nc.sync.dma_start(
                out=w1b_t[:], in_=w1b.rearrange("p (m k j) -> p m k j", m=4, k=4)
            )


            pz = {}  # pair -> [pz2_g0, pz2_g1], each [128, j, t, 256] (2 banks)
            st_tiles = {}
            thr_tiles = {}

            def dma_static(p):
                st = stp.tile([128, 4, 2, RPC], F32R, name=f"st{p}", tag="static")
                nc.sync.dma_start(
                    out=st[:],
                    in_=staticPM[p].rearrange("p (k t n) -> p k t n", k=4, t=2),
                )
                st_tiles[p] = st

            def dma_thr(t):
                thrB = thp.tile([128, 1, RPC], F32, name=f"thrB{t}", tag="thr")
                nc.scalar.dma_start(out=thrB[:, 0, :], in_=thrbc[t])
                thr_tiles[t] = thrB

            def mm_static_pair(p):
                """16 N=512 w1b matmuls covering steps (2p, 2p+1)."""
                st = st_tiles.pop(p)
                pzp_g = [
                    pzp.tile([128, 2, 2, RPC], F32, name=f"pz{p}_{g}", tag=f"pz{g}")
                    for g in range(2)
                ]
                pz[p] = pzp_g
                for g in range(2):
                    for j in range(2):
                        m = 2 * g + j
                        for k in range(4):
                            nc.tensor.matmul(
                                pzp_g[g][:, j, :, :],
                                w1b_t[:, m, k, :],
                                st[:, k, :, :],
                                start=(k == 0),
                                stop=False,
                            )

            def u_slice(u_pair, k):
                return u_pair[k // 2][:, k % 2, :]

            def emit_gate(t, s_pair, u_pair):
                """Identity blend + dynamic matmuls for step t into pz[t//2]."""
                pzt = pz[t // 2]
                ti = t % 2
                id_s = id_t[:, 2, :] if t == 0 else id_t[:, 0, :]
                for g in range(2):
                    for j in range(2):
                        nc.tensor.matmul(
                            pzt[g][:, j, ti, :],
                            id_s,
                            s_pair[g][:, j, :],
                            start=False,
                            stop=False,
                        )
                for g in range(2):
                    for j in range(2):
                        m = 2 * g + j
                        for k in range(4):
                            nc.tensor.matmul(
                                pzt[g][:, j, ti, :],
                                w1a_t[:, m, k, :],
                                u_slice(u_pair, k),
                                start=False,
                                stop=(ti == 1 and k == 3),
                            )

            # ---- prologue (order matters: w1b+st0 gate the first matmuls) ----
            dma_static(0)

            # step-0 gate inputs
            u0_g = []
            h0_g = []
            for g in range(2):
                u0 = wp.tile([128, 2, 256], F32R, name=f"u0_{g}")
                nc.sync.dma_start(
                    out=u0[:],
                    in_=u0T[256 * g : 256 * (g + 1), :].rearrange(
                        "(c p) n -> p c n", p=128
                    ),
                )
                u0_g.append(u0)
                h0t = wp.tile([128, 2, 256], F32R, name=f"h0_{g}")
                nc.sync.dma_start(
                    out=h0t[:],
                    in_=h0T[256 * g : 256 * (g + 1), :].rearrange(
                        "(c p) n -> p c n", p=128
                    ),
                )
                h0_g.append(h0t)
            id_t = wp.tile([128, 3, 128], F32R)
            nc.sync.dma_start(out=id_t[:], in_=idents[:])
            w1a_t = wp.tile([128, 4, 4, 128], F32R)
            nc.sync.dma_start(
                out=w1a_t[:], in_=w1a.rearrange("p (m k j) -> p m k j", m=4, k=4)
            )
            for p in range(1, PAIR_AHEAD):
                dma_static(p)
            for t in range(1, 3):
                dma_thr(t)
            mm_static_pair(0)
            emit_gate(0, h0_g, u0_g)

            h_prev = None
            for t in range(T):
                if t % 2 == 1:
                    p_new = (t + 1) // 2
                    if p_new + PAIR_AHEAD - 1 < PAIRS:
                        dma_static(p_new + PAIR_AHEAD - 1)
                    # next pair's static matmuls fill the sigma/u PE gap
                    if p_new < PAIRS:
                        mm_static_pair(p_new)
                if t + 3 < T:
                    dma_thr(t + 3)

                # sigma_t straight out of PSUM (strided read across both banks)
                s_g = [
                    sp.tile([128, 2, 256], F32R, name=f"s{t}_{g}", tag=f"s{g}")
                    for g in range(2)
                ]
                for g in range(2):
                    nc.scalar.activation(
                        s_g[g][:],
                        pz[t // 2][g][:, :, t % 2, :],
                        mybir.ActivationFunctionType.Sigmoid,
                    )

                # next step's gate: u_{t+1} = (s_t * DECAY) * thr_{t+1}
                if t + 1 < T:
                    thrB = thr_tiles.pop(t + 1)
                    u_g = [
                        up.tile([128, 2, 256], F32R, name=f"u{t}_{g}", tag=f"u{g}")
                        for g in range(2)
                    ]
                    for g in range(2):
                        nc.vector.scalar_tensor_tensor(
                            out=u_g[g][:],
                            in0=s_g[g][:],
                            scalar=DECAY,
                            in1=thrB[:].to_broadcast((128, 2, RPC)),
                            op0=mybir.AluOpType.mult,
                            op1=mybir.AluOpType.mult,
                        )
                    emit_gate(t + 1, s_g, u_g)

                # outputs: hd[:,0] = h_t = DECAY*s_t ; hd[:,1] = h_t - h_{t-1}
                hd_t = hp.tile([128, 2, 4, 256], F32, name=f"hd{t}", tag="hd")
                for g in range(2):
                    nc.vector.tensor_scalar_mul(
                        hd_t[:, 0, 2 * g : 2 * g + 2, :], s_g[g][:], DECAY
                    )
                if t >= 1:
                    for g in range(2):
                        nc.vector.scalar_tensor_tensor(
                            out=hd_t[:, 1, 2 * g : 2 * g + 2, :],
                            in0=s_g[g][:],
                            scalar=DECAY,
                            in1=h_prev[:, 0, 2 * g : 2 * g + 2, :],
                            op0=mybir.AluOpType.mult,
                            op1=mybir.AluOpType.subtract,
                        )
                    nc.sync.dma_start(
                        out=hdPM[t - 1].rearrange("p (w c n) -> p w c n", w=2, c=4),
                        in_=hd_t[:],
                    )
                h_prev = hd_t

    nc.compile()
    return nc


def get_program():
    global _prog
    if _prog is None:
        _prog = build_program()
    return _prog


def kernel(all_data_static, threshold_nc, all_data_dynamic_now, w1):
    all_data_static = np.asarray(all_data_static, dtype=np.float32)
    threshold_nc = np.asarray(threshold_nc, dtype=np.float32)
    h0 = np.asarray(all_data_dynamic_now, dtype=np.float32)
    w1 = np.asarray(w1, dtype=np.float32)

    nc = get_program()

    idents = np.zeros((128, 3, 128), dtype=np.float32)
    eye = np.eye(128, dtype=np.float32)
    idents[:, 0, :] = DECAY * eye
    idents[:, 1, :] = -eye
    idents[:, 2, :] = eye
    def wpm(w):
        # [512(k-major), 512(m-major)] -> [128(p), 4(m), 4(k), 128(j)]
        return np.ascontiguousarray(
            w.reshape(4, 128, 4, 128).transpose(1, 2, 0, 3)
        ).reshape(128, 4 * 4 * 128)

    w1a = wpm(w1[:DR2] - np.eye(DR2, dtype=np.float32))
    w1b = wpm(w1[DR2:])

    in_maps = []
    for c in range(CORES):
        r0 = c * RPC
        sl = slice(r0, r0 + RPC)
        thr_c = threshold_nc[:, sl, 0]  # [T, RPC]
        thr_bc = np.ascontiguousarray(
            np.broadcast_to(thr_c[:, None, :], (T, 128, RPC))
        )
        static_c = all_data_static[:, sl, :]  # [T, RPC, DR2]
        staticT = (static_c * thr_c[:, :, None]).transpose(0, 2, 1)  # [T, DR2, RPC]
        # partition-major pairs: [PAIRS, 128(p), 4(k), 2(t), RPC]
        staticPM = np.ascontiguousarray(
            staticT.reshape(PAIRS, 2, 4, 128, RPC).transpose(0, 3, 2, 1, 4)
        ).reshape(PAIRS, 128, 4 * 2 * RPC)
        h0_c = h0[sl]  # [RPC, DR2]
        u0T = np.ascontiguousarray((thr_c[0][:, None] * h0_c).T)
        h0Tc = np.ascontiguousarray(h0_c.T)
        in_maps.append(
            {
                "staticPM": staticPM,
                "thrbc": thr_bc,
                "u0T": u0T,
                "h0T": h0Tc,
                "w1a": w1a,
                "w1b": w1b,
                "idents": idents,
            }
        )

    trace = os.environ.get("KERNEL_TRACE", "0") == "1"
    res = run_bass_kernel_spmd(
        nc,
        in_maps,
        core_ids=list(range(CORES)),
        trace=trace,
        trace_cores=[0] if trace else None,
    )
    kernel.last_exec_time_ns = res.exec_time_ns
    kernel.last_results = res

    dynamic = np.empty((T, N, DR2), dtype=np.float32)
    diff = np.empty((T - 1, N, DR2), dtype=np.float32)
    dynamic[0] = h0
    for c in range(CORES):
        sl = slice(c * RPC, (c + 1) * RPC)
        out = res.results[c]
        # [63, 128(p), 2(w), 4(c), RPC] -> w-split -> [63, RPC, 4(c), 128(p)]
        hd = out["hdPM"].reshape(T - 1, 128, 2, 4, RPC)
        dynamic[1:, sl, :] = hd[:, :, 0].transpose(0, 3, 2, 1).reshape(T - 1, RPC, DR2)
        diff[:, sl, :] = hd[:, :, 1].transpose(0, 3, 2, 1).reshape(T - 1, RPC, DR2)
    final = dynamic[-1].copy()
    return dynamic, final, diff
